# revision 7
# baseline (speedup 1.0000x reference)
"""Decoder block (LN1 -> causal MHA -> LN2 -> GELU FFN, residuals) on 8 NeuronCores.

Sharding: 2-way data parallel over batch x 4-way tensor parallel over heads.
Core c: batch b=c//4, heads [4*(c%4) .. 4*(c%4)+4); after per-head chunked
AllToAlls of attention context, core c owns token slice [512*(c%4) .. +512)
of its batch for out-proj / LN2 / FFN.

v2 vs v1:
- bf16 weights + activations for every big matmul (FWL weight loads, half
  the weight DMA / SBUF / collective bytes); f32 PSUM accumulation.
- AllToAll split into 4 per-head chunks, overlapped with attention of the
  remaining heads; projections interleaved into head 0's query blocks.
- LN1 scale/shift broadcasts computed once per chunk (identical across
  feature chunks); squares on GpSimd instead of ACT.
- fc2 accumulates all 32 f-chunks in PSUM (8 banks, one per out chunk)
  instead of partial-sum round trips through SBUF.
"""
import sys
import numpy as np

sys.path.insert(0, '/opt/trn_rl_repo')

import ml_dtypes                        # noqa: E402
import concourse.bass as bass           # noqa: E402
import concourse.bacc as bacc           # noqa: E402
import concourse.tile as tile           # noqa: E402
from concourse import mybir             # noqa: E402
from concourse.masks import make_identity  # noqa: E402
from concourse.bass_utils import run_bass_kernel_spmd  # noqa: E402

F32 = mybir.dt.float32
F32R = mybir.dt.float32r
BF16 = mybir.dt.bfloat16
AF = mybir.ActivationFunctionType
ALU = mybir.AluOpType
BF = ml_dtypes.bfloat16

B, S, E, H, D, F = 2, 2048, 1024, 16, 64, 4096
NC = 8
T = S
TS = 512
EPS = 1e-5
NEH = E // 128         # 8
NFH = F // 128         # 32
HPC = 4                # heads per core
MCH = 2                # d-chunks for 4 heads
LW = 256               # layernorm / projection chunk width


def build(causal=True):
    nc = bacc.Bacc("TRN2", target_bir_lowering=False, debug=False, num_devices=NC)

    xT_d = nc.dram_tensor("xT", [E, T], BF16, kind="ExternalInput").ap()
    wq_d = nc.dram_tensor("wq", [E, HPC * D], BF16, kind="ExternalInput").ap()
    wk_d = nc.dram_tensor("wk", [E, HPC * D], BF16, kind="ExternalInput").ap()
    wv_d = nc.dram_tensor("wv", [E, HPC * D], BF16, kind="ExternalInput").ap()
    bq_d = nc.dram_tensor("bq", [128, MCH], F32, kind="ExternalInput").ap()
    bk_d = nc.dram_tensor("bk", [128, MCH], F32, kind="ExternalInput").ap()
    bv_d = nc.dram_tensor("bv", [1, HPC * D], BF16, kind="ExternalInput").ap()
    wot_d = nc.dram_tensor("wot", [NEH, 128, E], BF16, kind="ExternalInput").ap()
    bo_d = nc.dram_tensor("bo", [128, NEH], F32, kind="ExternalInput").ap()
    w1_d = nc.dram_tensor("w1", [NFH, 128, E], BF16, kind="ExternalInput").ap()
    b1_d = nc.dram_tensor("b1", [128, NFH], F32, kind="ExternalInput").ap()
    w2_d = nc.dram_tensor("w2", [F, E], BF16, kind="ExternalInput").ap()
    b2_d = nc.dram_tensor("b2", [128, NEH], F32, kind="ExternalInput").ap()
    g2r_d = nc.dram_tensor("g2r", [1, E], F32R, kind="ExternalInput").ap()
    b2r_d = nc.dram_tensor("b2r", [1, E], F32R, kind="ExternalInput").ap()
    xres_d = nc.dram_tensor("xres", [E, TS], F32, kind="ExternalInput").ap()
    zm_d = nc.dram_tensor("zm", [128, NC], F32, kind="ExternalInput").ap()
    oncf_d = nc.dram_tensor("oncf", [128, 1], F32R, kind="ExternalInput").ap()
    oncb_d = nc.dram_tensor("oncb", [128, 1], BF16, kind="ExternalInput").ap()
    onrf_d = nc.dram_tensor("onrf", [1, 128], F32R, kind="ExternalInput").ap()
    onrb_d = nc.dram_tensor("onrb", [1, 128], BF16, kind="ExternalInput").ap()
    on4_d = nc.dram_tensor("on4", [128, HPC], BF16, kind="ExternalInput").ap()
    on5_d = nc.dram_tensor("on5", [1, TS], F32R, kind="ExternalInput").ap()
    if not causal:
        mT_d = nc.dram_tensor("mT", [S, S], F32, kind="ExternalInput").ap()
    out_d = nc.dram_tensor("out", [TS, E], F32, kind="ExternalOutput").ap()

    with tile.TileContext(nc) as tc:
        with (
            tc.tile_pool(name="res", bufs=1) as res,
            tc.tile_pool(name="wk_", bufs=2) as wrk,
            tc.tile_pool(name="st", bufs=1) as st,
            tc.tile_pool(name="ps", bufs=2, space="PSUM") as ps,
            tc.tile_pool(name="dram", bufs=1, space="DRAM") as dram,
        ):
            def rtile(name, shape, dt, tag=None):
                return res.tile(shape, dt, tag=tag or name, name=name)

            def pA(name):
                return ps.tile([128, 1024], F32, tag="pA", name=name, bufs=2)

            def pB(name, shape=(128, 512)):
                return ps.tile(list(shape), F32, tag="pB", name=name, bufs=4)

            # ---- constants ----
            oncf = rtile("oncf", [128, 1], F32R)
            nc.sync.dma_start(oncf[:], oncf_d[:])
            oncb = rtile("oncb", [128, 1], BF16)
            nc.sync.dma_start(oncb[:], oncb_d[:])
            onrf = rtile("onrf", [1, 128], F32R)
            nc.sync.dma_start(onrf[:], onrf_d[:])
            onrb = rtile("onrb", [1, 128], BF16)
            nc.sync.dma_start(onrb[:], onrb_d[:])
            on4 = rtile("on4", [128, HPC], BF16)
            nc.sync.dma_start(on4[:], on4_d[:])
            on5 = rtile("on5", [1, TS], F32R)
            nc.sync.dma_start(on5[:], on5_d[:])
            ident = rtile("ident", [128, 128], F32)
            make_identity(nc, ident[:])
            zm = rtile("zm", [128, NC], F32)
            nc.sync.dma_start(zm[:], zm_d[:])
            bq = rtile("bq", [128, MCH], F32)
            nc.sync.dma_start(bq[:], bq_d[:])
            bk = rtile("bk", [128, MCH], F32)
            nc.sync.dma_start(bk[:], bk_d[:])
            bv = rtile("bv", [1, HPC * D], BF16)
            nc.sync.dma_start(bv[:], bv_d[:])
            bo = rtile("bo", [128, NEH], F32)
            nc.sync.dma_start(bo[:], bo_d[:])
            b1 = rtile("b1", [128, NFH], F32)
            nc.sync.dma_start(b1[:], b1_d[:])
            b2 = rtile("b2", [128, NEH], F32)
            nc.sync.dma_start(b2[:], b2_d[:])

            # ---- resident weights/tensors ----
            wqb = rtile("wqb", [128, NEH * HPC * D], BF16)      # [128, 2048]
            for e in range(NEH):
                nc.sync.dma_start(wqb[:, 256 * e:256 * e + 256],
                                  wq_d[128 * e:128 * e + 128, :])
            wkvb = rtile("wkvb", [128, 4096], BF16)
            for e in range(NEH):
                nc.sync.dma_start(wkvb[:, 256 * e:256 * e + 256],
                                  wk_d[128 * e:128 * e + 128, :])
                nc.sync.dma_start(wkvb[:, 2048 + 256 * e:2048 + 256 * e + 256],
                                  wv_d[128 * e:128 * e + 128, :])
            qtb = rtile("qtb", [128, MCH * T], BF16)            # [128, 4096]
            ktb = rtile("ktb", [128, MCH * T], BF16)            # [128, 4096]
            vpb = rtile("vpb", [128, (T // 128) * HPC * 65], BF16)

            def vsl(tt):
                return vpb[:, 260 * tt:260 * tt + 260]

            # ---- layernorm standardization helper ----
            # Computes per-token scale a=rstd, shift c=-mean*rstd rank-1
            # broadcast tiles.  For LN1 (fold_g False) one (bca,bcc) pair per
            # chunk (identical across feature chunks); for LN2 a per-e pair
            # with gamma/beta folded.  put_out(e, bca_ap, bcc_ap).
            def ln_chunk(get_src, width, put_out, idx, fold_g=False,
                         src_bf16=True):
                sq_dt = BF16 if src_bf16 else F32R
                onc = oncb if src_bf16 else oncf
                sq_l = []
                for e in range(NEH):
                    sq = wrk.tile([128, width], sq_dt, tag="sq", name=f"sq{idx}_{e}",
                                  bufs=3)
                    if src_bf16:
                        nc.gpsimd.tensor_tensor(sq[:], get_src(e), get_src(e),
                                                ALU.mult)
                    else:
                        nc.gpsimd.tensor_tensor(sq[:],
                                                get_src(e).bitcast(F32),
                                                get_src(e).bitcast(F32), ALU.mult)
                    psq = (pB(f"psq{idx}", (1, width)) if e == 0 else psq)
                    nc.tensor.matmul(psq[:], onc[:], sq[:],
                                     start=(e == 0), stop=(e == NEH - 1))
                    sq_l.append(sq)
                psx = pB(f"psx{idx}", (1, width))
                for e in range(NEH):
                    nc.tensor.matmul(psx[:], onc[:], get_src(e),
                                     start=(e == 0), stop=(e == NEH - 1))
                mean = st.tile([1, width], F32, tag="mean", name=f"mean{idx}")
                nc.vector.tensor_scalar_mul(mean[:], psx[:], 1.0 / E)
                msq = st.tile([1, width], F32, tag="msq", name=f"msq{idx}")
                nc.vector.tensor_scalar_mul(msq[:], psq[:], 1.0 / E)
                scr = st.tile([1, width], F32, tag="scr", name=f"scr{idx}")
                nc.vector.tensor_tensor(scr[:], mean[:], mean[:], ALU.mult)
                var = st.tile([1, width], F32, tag="var", name=f"var{idx}")
                nc.vector.scalar_tensor_tensor(
                    out=var[:], in0=msq[:], scalar=float(EPS), in1=scr[:],
                    op0=ALU.add, op1=ALU.subtract)
                std = st.tile([1, width], F32, tag="std", name=f"std{idx}")
                nc.scalar.activation(std[:], var[:], AF.Sqrt)
                rstd = st.tile([1, width], F32, tag="rstd", name=f"rstd{idx}")
                nc.vector.reciprocal_approx_fast(rstd[:], std[:])
                a_row = st.tile([1, width], F32R, tag="a_row", name=f"a_row{idx}")
                nc.vector.tensor_copy(a_row[:], rstd[:])
                ct = st.tile([1, width], F32, tag="ct", name=f"ct{idx}")
                nc.vector.tensor_tensor(ct[:], mean[:], rstd[:], ALU.mult)
                c_row = st.tile([1, width], F32R, tag="c_row", name=f"c_row{idx}")
                nc.vector.tensor_scalar_mul(c_row[:], ct[:], -1.0)
                if not fold_g:
                    bca = pB(f"bca{idx}", (128, width))
                    bcc = pB(f"bcc{idx}", (128, width))
                    nc.tensor.matmul(bca[:], onrf[:], a_row[:],
                                     start=True, stop=True)
                    nc.tensor.matmul(bcc[:], onrf[:], c_row[:],
                                     start=True, stop=True)
                    bca_s = wrk.tile([128, width], BF16, tag="bcs",
                                     name=f"bcas{idx}", bufs=4)
                    nc.vector.tensor_copy(bca_s[:], bca[:])
                    bcc_s = wrk.tile([128, width], BF16, tag="bcs",
                                     name=f"bccs{idx}", bufs=4)
                    nc.vector.tensor_copy(bcc_s[:], bcc[:])
                    for e in range(NEH):
                        put_out(e, bca_s, bcc_s)
                else:
                    for e in range(NEH):
                        bca = pB(f"bca{idx}_{e}", (128, width))
                        bcc = pB(f"bcc{idx}_{e}", (128, width))
                        g2s = wrk.tile([1, 128], F32R, tag="g2s",
                                       name=f"g2s{idx}_{e}", bufs=4)
                        nc.sync.dma_start(g2s[:], g2r_d[:, 128 * e:128 * e + 128])
                        b2s = wrk.tile([1, 128], F32R, tag="g2s",
                                       name=f"b2s{idx}_{e}", bufs=4)
                        nc.sync.dma_start(b2s[:], b2r_d[:, 128 * e:128 * e + 128])
                        nc.tensor.matmul(bca[:], g2s[:], a_row[:],
                                         start=True, stop=True)
                        nc.tensor.matmul(bcc[:], g2s[:], c_row[:],
                                         start=True, stop=False)
                        nc.tensor.matmul(bcc[:], b2s[:], on5[:, 0:width],
                                         start=False, stop=True)
                        put_out(e, bca, bcc)

            # ---- phase A: LN1 fused with Q/K/V projections, per 256 tokens --
            def proj_chunk(tcn):
                xt_l = []
                for e in range(NEH):
                    xt = wrk.tile([128, LW], BF16, tag="xt", name=f"xt{tcn}_{e}",
                                  bufs=9)
                    nc.sync.dma_start(
                        xt[:], xT_d[128 * e:128 * e + 128, LW * tcn:LW * tcn + LW])
                    xt_l.append(xt)
                zz_l = []

                def put_z(e, bca_s, bcc_s):
                    tmp = wrk.tile([128, LW], BF16, tag="lnt", name=f"lnt{tcn}_{e}",
                                   bufs=3)
                    eng = nc.gpsimd if (e % 2) else nc.vector
                    eng.tensor_tensor(tmp[:], xt_l[e][:], bca_s[:], ALU.mult)
                    zz = wrk.tile([128, LW], BF16, tag="zz", name=f"zz{tcn}_{e}",
                                  bufs=9)
                    eng.tensor_tensor(zz[:], tmp[:], bcc_s[:], ALU.add)
                    zz_l.append(zz)

                ln_chunk(lambda e: xt_l[e][:], LW, put_z, f"l1c{tcn}")

                for m in range(MCH):
                    pq = pB(f"pq{tcn}_{m}", (128, LW))
                    pk = pB(f"pk{tcn}_{m}", (128, LW))
                    for e in range(NEH):
                        nc.tensor.matmul(
                            pq[:], wqb[:, 256 * e + 128 * m:256 * e + 128 * m + 128],
                            zz_l[e][:], start=(e == 0), stop=(e == NEH - 1))
                    for e in range(NEH):
                        nc.tensor.matmul(
                            pk[:], wkvb[:, 256 * e + 128 * m:256 * e + 128 * m + 128],
                            zz_l[e][:], start=(e == 0), stop=(e == NEH - 1))
                    c0 = 2048 * m + LW * tcn
                    nc.vector.tensor_scalar_add(qtb[:, c0:c0 + LW], pq[:],
                                                bq[:, m:m + 1])
                    nc.vector.tensor_scalar_add(ktb[:, c0:c0 + LW], pk[:],
                                                bk[:, m:m + 1])
                for sub in range(LW // 128):
                    tt = (LW * tcn) // 128 + sub
                    pv = pB(f"pv{tt}", (128, HPC * D))
                    for e in range(NEH):
                        nc.tensor.matmul(
                            pv[:], zz_l[e][:, 128 * sub:128 * sub + 128],
                            wkvb[:, 2048 + 256 * e:2048 + 256 * e + 256],
                            start=(e == 0), stop=False)
                    nc.tensor.matmul(pv[:], onrb[:], bv[:], start=False, stop=True)
                    nc.vector.tensor_copy(
                        vsl(tt).rearrange("p (h d) -> p h d", h=HPC)[:, :, 0:D],
                        pv[:].rearrange("p (h d) -> p h d", h=HPC))
                    nc.vector.tensor_copy(
                        vsl(tt).rearrange("p (h d) -> p h d", h=HPC)[:, :, D:D + 1],
                        on4[:].rearrange("p (h o) -> p h o", o=1))

            # ---- attention for one (head, query-block) ----
            stage_l = [dram.tile([NC, D, 512], BF16, tag=f"stg{h}",
                                 name=f"stage{h}") for h in range(HPC)]
            stage2_l = [dram.tile([NC, D, 512], BF16, tag=f"st2{h}",
                                  name=f"stage2{h}") for h in range(HPC)]
            NSB = T // 512

            def attn_block(h, i):
                m, hp = h // 2, h % 2
                qsl = qtb[64 * hp:64 * hp + 64, 2048 * m + 512 * i:
                          2048 * m + 512 * i + 512]
                n_kc = 4 * (i + 1) if causal else T // 128
                psav = pB(f"av{h}_{i}")
                ngr = (n_kc + 1) // 2
                psc_l = {}

                def emit_qk(g):
                    kcs = [k for k in (2 * g, 2 * g + 1) if k < n_kc]
                    psc = pA(f"sc{h}_{i}_{g}")
                    for u, kc in enumerate(kcs):
                        nc.tensor.matmul(
                            psc[:, 512 * u:512 * u + 512],
                            ktb[64 * hp:64 * hp + 64,
                                2048 * m + 128 * kc:2048 * m + 128 * kc + 128],
                            qsl, start=True, stop=True)
                    if not causal:
                        for u, kc in enumerate(kcs):
                            mb = wrk.tile([128, 512], F32, tag="mb",
                                          name=f"mb{h}_{i}_{g}_{u}", bufs=2)
                            nc.sync.dma_start(
                                mb[:], mT_d[128 * kc:128 * kc + 128,
                                            512 * i:512 * i + 512])
                            nc.vector.tensor_tensor(
                                psc[:, 512 * u:512 * u + 512],
                                psc[:, 512 * u:512 * u + 512], mb[:], ALU.add)
                    psc_l[g] = (psc, kcs)

                emit_qk(0)
                for g in range(ngr):
                    # pipeline: next group's QK matmuls go ahead of this
                    # group's AV so the exp (ACT) latency hides behind PE
                    if g + 1 < ngr:
                        emit_qk(g + 1)
                    psc, kcs = psc_l.pop(g)
                    eg = wrk.tile([128, 1024], BF16, tag="exp",
                                  name=f"exp{h}_{i}_{g}", bufs=3)
                    w = 512 * len(kcs)
                    nc.scalar.activation(eg[:, 0:w], psc[:, 0:w], AF.Exp)
                    if causal:
                        for u, kc in enumerate(kcs):
                            j = kc - 4 * i
                            if j >= 0:
                                nc.gpsimd.affine_select(
                                    out=eg[:, 512 * u:512 * u + 512],
                                    in_=eg[:, 512 * u:512 * u + 512],
                                    compare_op=ALU.is_ge, fill=0.0,
                                    base=-128 * j, pattern=[[1, 512]],
                                    channel_multiplier=-1)
                    for u, kc in enumerate(kcs):
                        nc.tensor.matmul(
                            psav[0:65, :], vsl(kc)[:, 65 * h:65 * h + 65],
                            eg[:, 512 * u:512 * u + 512],
                            start=(kc == 0), stop=(kc == n_kc - 1),
                            skip_group_check=True)
                sden = st.tile([1, 512], F32, tag="sden", name=f"sden{h}_{i}")
                nc.scalar.copy(sden[:], psav[64:65, :])
                rc = st.tile([1, 512], F32, tag="rc", name=f"rc{h}_{i}")
                nc.vector.reciprocal_approx_fast(rc[:], sden[:])
                rb = wrk.tile([64, 512], F32, tag="rb", name=f"rb{h}_{i}", bufs=2)
                nc.gpsimd.partition_broadcast(rb[:], rc[:])
                ctx = wrk.tile([64, 512], BF16, tag="ctx", name=f"ctx{h}_{i}",
                               bufs=2)
                nc.vector.tensor_tensor(ctx[:], psav[0:64, :], rb[:], ALU.mult)
                for s_ in (i, i + 4):
                    cz = wrk.tile([64, 512], BF16, tag="cz",
                                  name=f"cz{h}_{i}_{s_}", bufs=3)
                    nc.vector.tensor_scalar_mul(cz[:], ctx[:], zm[0:64, s_:s_ + 1])
                    nc.sync.dma_start(stage_l[h][s_], cz[:])

            # cfb: attention context gathered for this core's 512 tokens,
            # E laid out in global-head order (head g rows at
            # [(g%2)*64, +64) of column chunk g//2).
            cfb = rtile("cfb", [128, 4096], BF16)

            def gather_head(h):
                nc.gpsimd.collective_compute(
                    "AllToAll", ALU.bypass, replica_groups=[list(range(NC))],
                    ins=[stage_l[h].opt()], outs=[stage2_l[h].opt()])
                for j in range(HPC):
                    g = 4 * j + h
                    sa = wrk.tile([64, 512], BF16, tag="sa", name=f"sa{h}_{j}",
                                  bufs=4)
                    nc.sync.dma_start(sa[:], stage2_l[h][j])
                    sb_ = wrk.tile([64, 512], BF16, tag="sa", name=f"sb{h}_{j}",
                                   bufs=4)
                    nc.sync.dma_start(sb_[:], stage2_l[h][j + 4])
                    r0 = 64 * (g % 2)
                    c0 = 512 * (g // 2)
                    nc.vector.tensor_tensor(cfb[r0:r0 + 64, c0:c0 + 512],
                                            sa[:], sb_[:], ALU.add)

            # ---- schedule: projections interleaved into head 0, then the
            # remaining heads with per-head chunked AllToAlls ----
            for tcn in range(T // LW):
                proj_chunk(tcn)
                if causal and tcn % 2 == 1:
                    attn_block(0, tcn // 2)
            if not causal:
                for i in range(NSB):
                    attn_block(0, i)
            gather_head(0)
            for h in range(1, HPC):
                for i in range(NSB):
                    attn_block(h, i)
                gather_head(h)

            # ---- phase C: out-proj + residual + LN2 ----
            yb = rtile("yb", [128, 4096], F32R)
            for eo in range(NEH):
                wof = wrk.tile([128, E], BF16, tag="ws", name=f"wof{eo}", bufs=2)
                nc.sync.dma_start(wof[:], wot_d[eo])
                po = pB(f"po{eo}")
                for ee in range(NEH):
                    nc.tensor.matmul(po[:], wof[:, 128 * ee:128 * ee + 128],
                                     cfb[:, 512 * ee:512 * ee + 512],
                                     start=(ee == 0), stop=(ee == NEH - 1))
                xrs = wrk.tile([128, 512], F32, tag="xrs", name=f"xrs{eo}", bufs=2)
                nc.sync.dma_start(xrs[:], xres_d[128 * eo:128 * eo + 128, :])
                nc.vector.scalar_tensor_tensor(
                    out=yb[:, 512 * eo:512 * eo + 512], in0=po[:],
                    scalar=bo[:, eo:eo + 1], in1=xrs[:], op0=ALU.add, op1=ALU.add)
            x2r = rtile("x2rb", [128, 4096], BF16)
            for lc in range(TS // LW):
                def put_x2(e, bca, bcc, lc=lc):
                    tmp = wrk.tile([128, LW], F32, tag="ln2t", name=f"lnt2{lc}_{e}",
                                   bufs=3)
                    eng = nc.vector
                    eng.tensor_tensor(
                        tmp[:],
                        yb[:, 512 * e + LW * lc:512 * e + LW * lc + LW].bitcast(F32),
                        bca[:], ALU.mult)
                    eng.tensor_tensor(
                        x2r[:, 512 * e + LW * lc:512 * e + LW * lc + LW],
                        tmp[:], bcc[:], ALU.add)

                ln_chunk(
                    lambda e, lc=lc:
                        yb[:, 512 * e + LW * lc:512 * e + LW * lc + LW],
                    LW, put_x2, f"l2c{lc}", fold_g=True, src_bf16=False)

            # ---- phase D: FFN. fc1 -> resident h1, then fc2 with all 32
            # f-chunks accumulated in PSUM (one bank per output chunk) ----
            h1b = rtile("h1b", [128, NFH * 512], BF16)          # [128, 16384]
            for f in range(NFH):
                w1f = wrk.tile([128, E], BF16, tag="ws", name=f"w1f{f}", bufs=2)
                nc.sync.dma_start(w1f[:], w1_d[f])
                pf = pB(f"pf{f}")
                for e in range(NEH):
                    nc.tensor.matmul(pf[:], w1f[:, 128 * e:128 * e + 128],
                                     x2r[:, 512 * e:512 * e + 512],
                                     start=(e == 0), stop=(e == NEH - 1))
                nc.scalar.activation(h1b[:, 512 * f:512 * f + 512], pf[:],
                                     AF.Gelu, bias=b1[:, f:f + 1])
            outT = rtile("outT", [128, 4096], F32)
            for half in range(2):
                p2_l = [pB(f"p2_{half}_{q}") for q in range(4)]
                for f in range(NFH):
                    w2f = wrk.tile([128, 512], BF16, tag="w2s", name=f"w2f{half}_{f}",
                                   bufs=3)
                    nc.sync.dma_start(
                        w2f[:], w2_d[128 * f:128 * f + 128,
                                     512 * half:512 * half + 512])
                    for q in range(4):
                        nc.tensor.matmul(p2_l[q][:],
                                         w2f[:, 128 * q:128 * q + 128],
                                         h1b[:, 512 * f:512 * f + 512],
                                         start=(f == 0), stop=(f == NFH - 1),
                                         skip_group_check=True)
                for q in range(4):
                    eo = 4 * half + q
                    x2f = wrk.tile([128, 512], F32, tag="x2f", name=f"x2f{eo}",
                                   bufs=2)
                    nc.vector.tensor_copy(x2f[:], x2r[:, 512 * eo:512 * eo + 512])
                    nc.vector.scalar_tensor_tensor(
                        out=outT[:, 512 * eo:512 * eo + 512],
                        in0=p2_l[q][:],
                        scalar=b2[:, eo:eo + 1], in1=x2f[:],
                        op0=ALU.add, op1=ALU.add)

            # ---- phase E: transpose to [tokens, E] and store ----
            for ts_ in range(TS // 128):
                osb = wrk.tile([128, E], F32, tag="osb", name=f"osb{ts_}", bufs=2)
                for eo in range(NEH):
                    pt = pB(f"pt{ts_}_{eo}", (128, 128))
                    nc.tensor.transpose(
                        pt[:], outT[:, 512 * eo + 128 * ts_:512 * eo + 128 * ts_ + 128],
                        ident[:])
                    nc.vector.tensor_copy(osb[:, 128 * eo:128 * eo + 128], pt[:])
                nc.sync.dma_start(out_d[128 * ts_:128 * ts_ + 128, :], osb[:])

    nc.compile()
    return nc


_CACHE = {}


def _get_nc(causal):
    if causal not in _CACHE:
        _CACHE[causal] = build(causal)
    return _CACHE[causal]


def _build_in_maps(x, mask, Wq, bq, Wk, bk, Wv, bv, Wo, bo, W1, b1, W2, b2,
                   ln1_g, ln1_b, ln2_g, ln2_b):
    x = np.asarray(x, np.float32)
    mask2d = np.asarray(mask).reshape(S, S)
    causal = bool(np.array_equal(mask2d, np.tril(np.ones((S, S), mask2d.dtype))))

    def colmaj(v, nch):
        return np.ascontiguousarray(np.asarray(v, np.float32).reshape(nch, 128).T)

    ln1_g = np.asarray(ln1_g, np.float32)
    ln1_b = np.asarray(ln1_b, np.float32)
    Wq = np.asarray(Wq, np.float32)
    Wk = np.asarray(Wk, np.float32)
    Wv = np.asarray(Wv, np.float32)
    Wo = np.asarray(Wo, np.float32)
    W1 = np.asarray(W1, np.float32)
    Wqf = Wq * ln1_g[:, None] / np.sqrt(D)
    bqf = (ln1_b @ Wq + np.asarray(bq, np.float32)) / np.sqrt(D)
    Wkf = Wk * ln1_g[:, None]
    bkf = ln1_b @ Wk + np.asarray(bk, np.float32)
    Wvf = Wv * ln1_g[:, None]
    bvf = ln1_b @ Wv + np.asarray(bv, np.float32)
    w1t = np.ascontiguousarray(
        W1.reshape(NEH, 128, NFH, 128).transpose(2, 1, 0, 3).reshape(NFH, 128, E)
    ).astype(BF)
    wot = np.ascontiguousarray(
        Wo.reshape(NEH, 128, NEH, 128).transpose(2, 1, 0, 3).reshape(NEH, 128, E)
    ).astype(BF)

    xT = [np.ascontiguousarray(x[b_].T).astype(BF) for b_ in range(B)]
    xTf = [np.ascontiguousarray(x[b_].T) for b_ in range(B)]
    shared = {
        "wot": wot, "bo": colmaj(bo, NEH),
        "w1": w1t, "b1": colmaj(b1, NFH),
        "w2": np.ascontiguousarray(np.asarray(W2, np.float32)).astype(BF),
        "b2": colmaj(b2, NEH),
        "g2r": np.ascontiguousarray(np.asarray(ln2_g, np.float32)[None, :]),
        "b2r": np.ascontiguousarray(np.asarray(ln2_b, np.float32)[None, :]),
        "oncf": np.ones((128, 1), np.float32),
        "oncb": np.ones((128, 1), BF),
        "onrf": np.ones((1, 128), np.float32),
        "onrb": np.ones((1, 128), BF),
        "on4": np.ones((128, HPC), BF),
        "on5": np.ones((1, TS), np.float32),
    }
    if not causal:
        shared["mT"] = np.ascontiguousarray(
            np.where(mask2d == 0, np.float32(-1e9), np.float32(0.0)).T)
    zms = []
    for b_ in range(B):
        z_ = np.zeros((128, NC), np.float32)
        z_[:, 4 * b_:4 * b_ + 4] = 1.0
        zms.append(z_)

    in_maps = []
    for c in range(NC):
        b_, j = c // 4, c % 4
        cs = slice(HPC * D * j, HPC * D * (j + 1))
        m = {
            "xT": xT[b_], "zm": zms[b_],
            "xres": np.ascontiguousarray(xTf[b_][:, TS * j:TS * (j + 1)]),
            "wq": np.ascontiguousarray(Wqf[:, cs]).astype(BF),
            "wk": np.ascontiguousarray(Wkf[:, cs]).astype(BF),
            "wv": np.ascontiguousarray(Wvf[:, cs]).astype(BF),
            "bq": colmaj(bqf[cs], MCH),
            "bk": colmaj(bkf[cs], MCH),
            "bv": np.ascontiguousarray(bvf[None, cs]).astype(BF),
        }
        m.update(shared)
        in_maps.append(m)
    return in_maps, causal


def kernel(**inputs):
    in_maps, causal = _build_in_maps(**inputs)
    nc_obj = _get_nc(causal)
    res = run_bass_kernel_spmd(nc_obj, in_maps, list(range(NC)))
    out = np.empty((B, S, E), np.float32)
    for c in range(NC):
        b_, j = c // 4, c % 4
        out[b_, TS * j:TS * (j + 1), :] = res.results[c]["out"]
    return out
